# revision 17
# baseline (speedup 1.0000x reference)
"""Trainium2 Bass kernel for nn_CellSmooth.

Computes: out = softmax(-cdist(enc, enc) + quality^T, axis=-1) @ expression
for B=1, N=8192, G=2048, D=64, sharded row-wise across 8 NeuronCores.

Math/layout design (per core, owning a 1024-row block of queries i):
  * d2[j, i] = |e_j|^2 + |e_i|^2 - 2 e_j.e_i is produced TRANSPOSED ([j, i]
    tiles, j on partitions) by a single K=66 augmented matmul:
      U[:, j] = [enc_j (64), |e_j|^2, 1],  V[:, i] = [-2 enc_i (64), 1, |e_i|^2]
    U/V are built on the host (tiny).
  * Inputs are j-rotated per core on the host (roll by -1024*c) so each
    core's diagonal block lands at compile-time-known j-tiles; those run in
    fp32 (the d2_ii ~ 0 cancellation needs it) with a relu; all other j-tiles
    run float32r at full PE rate, and their d2 >= ~30 so sqrt is safe
    straight out of PSUM. The softmax sum over j is permutation invariant,
    so the rotation does not change the output.
  * P^T[j, i] = exp(quality_j - sqrt(d2)) via ACT; quality folds into the ACT
    exp bias (per-partition, j on partitions). sqrt and exp live in different
    ACT table sets, so tiles are processed in groups of G=[sqrt x G, exp x G]
    to amortize table swaps.
  * The [j, i] P^T layout is exactly the stationary-operand layout the output
    matmul needs - the big NxN matrix is never transposed.
  * denominator_i = sum_j P^T[j, i] via a ones-stationary matmul column-sum,
    redistributed [1, 512] -> [128, 4] through a DRAM bounce.
  * out[i, g] = (sum_j P^T[j, i] E[j, g]) / den_i: 64 j-tile accumulation in
    PSUM with float32r operands (full PE rate at N=512), one DVE
    tensor_scalar multiply by 1/den, DMA out.
  * i is processed in two 512-column halves so P^T fits in SBUF; expression
    streams from HBM once per half (2KB/partition contiguous DMAs).
"""

import contextlib

import numpy as np

import concourse.bass as bass  # noqa: F401
import concourse.mybir as mybir
import concourse.tile as tile
from concourse import bacc

F32 = mybir.dt.float32
F32R = mybir.dt.float32r
AF = mybir.ActivationFunctionType

P = 128
N_CORES = 8


def build_nc(n=8192, d=64, rows=1024, g=2048, half=512, use_f32r=True, repeat=1,
             hw_loop=0):
    """Build the per-core Bass program (SPMD: all per-core variation is in
    the input data, never in the instruction stream).

    repeat>1 re-runs the body unrolled; hw_loop>0 wraps the body in a
    hardware For_i loop (constant instruction count). Both are for measuring
    steady-state HW exec time by wall-clock differencing in test.py.
    """
    jt_n = n // P          # j tiles (contraction for the output matmul)
    n_half = rows // half  # i column passes
    it_n = half // P       # i tiles per pass
    gb_n = g // 512        # g blocks
    k = d + 2              # augmented contraction for the d2 matmul
    mm_dt = F32R if use_f32r else F32

    nc = bacc.Bacc(None, target_bir_lowering=False)
    u_d = nc.dram_tensor("u", [k, n], F32, kind="ExternalInput")
    v_d = nc.dram_tensor("v", [k, rows], F32, kind="ExternalInput")
    qt_d = nc.dram_tensor("qt", [P, jt_n], F32, kind="ExternalInput")
    e_d = nc.dram_tensor("expr", [n, g], F32, kind="ExternalInput")
    o_d = nc.dram_tensor("out", [rows, g], F32, kind="ExternalOutput")

    with tile.TileContext(nc) as tc:
        with (
            tc.tile_pool(name="const", bufs=1) as constp,
            tc.tile_pool(name="vpool", bufs=2) as vpool,
            tc.tile_pool(name="ptpool", bufs=1) as ptpool,
            tc.tile_pool(name="estream", bufs=4) as epool,
            tc.tile_pool(name="ostage", bufs=4) as opool,
            tc.tile_pool(name="small", bufs=2) as smallp,
            tc.tile_pool(name="mmpsum", bufs=7, space="PSUM") as mmpsum,
            tc.tile_pool(name="denpsum", bufs=1, space="PSUM") as denpsum,
            tc.tile_pool(name="scratch", bufs=2, space="DRAM") as dramp,
        ):
            u_sb = constp.tile([k, n], mm_dt, name="u_sb")
            nc.sync.dma_start(out=u_sb, in_=u_d[:, :].bitcast(mm_dt))
            qt_sb = constp.tile([P, jt_n], F32, name="qt_sb")
            nc.sync.dma_start(out=qt_sb, in_=qt_d[:, :])
            ones_f32 = constp.tile([P, 1], F32, name="ones_f32")
            nc.vector.memset(ones_f32, 1.0)
            ones_sb = constp.tile([P, 1], mm_dt, name="ones_sb")
            nc.vector.tensor_copy(out=ones_sb[:, :], in_=ones_f32[:, :])

            def body():
                for h in [hh for _ in range(repeat) for hh in range(n_half)]:
                    v_sb = vpool.tile([k, half], mm_dt, name="v_sb")
                    nc.sync.dma_start(
                        out=v_sb,
                        in_=v_d[:, h * half:(h + 1) * half].bitcast(mm_dt))

                    pt = ptpool.tile([P, jt_n, half], mm_dt, name="pt",
                                     tag="pt")

                    # ---- phase 1: P^T tiles ----
                    diag_lo, diag_hi = h * it_n, (h + 1) * it_n
                    G = 8
                    for jg in range(0, jt_n, G):
                        for j in range(jg, min(jg + G, jt_n)):
                            diag = diag_lo <= j < diag_hi
                            d2 = mmpsum.tile([P, half], F32, name="d2",
                                             tag="mm")
                            u_ap = u_sb[:, j * P:(j + 1) * P]
                            v_ap = v_sb[:, :]
                            if diag:
                                u_ap = u_ap.bitcast(F32)
                                v_ap = v_ap.bitcast(F32)
                            nc.tensor.matmul(d2[:, :], u_ap, v_ap,
                                             start=True, stop=True)
                            ptj = pt[:, j, :]
                            if diag:
                                nc.vector.tensor_scalar_max(
                                    out=ptj, in0=d2[:, :], scalar1=0.0)
                                nc.scalar.activation(out=ptj, in_=ptj,
                                                     func=AF.Sqrt)
                            else:
                                nc.scalar.activation(out=ptj, in_=d2[:, :],
                                                     func=AF.Sqrt)
                        for j in range(jg, min(jg + G, jt_n)):
                            ptj = pt[:, j, :]
                            nc.scalar.activation(
                                out=ptj, in_=ptj, func=AF.Exp,
                                bias=qt_sb[:, j:j + 1], scale=-1.0,
                            )

                    # ---- softmax denominators (column sums of P^T) ----
                    den_ps = denpsum.tile([1, half], F32, name="den_ps",
                                          tag="den")
                    for j in range(jt_n):
                        nc.tensor.matmul(
                            den_ps[:, :], ones_sb[:, :], pt[:, j, :],
                            start=(j == 0), stop=(j == jt_n - 1),
                        )
                    den_row = smallp.tile([1, half], F32, name="den_row")
                    nc.vector.tensor_copy(out=den_row[:, :], in_=den_ps[:, :])
                    den_dram = dramp.tile([1, half], F32, name="den_dram")
                    nc.sync.dma_start(out=den_dram[:, :], in_=den_row[:, :])
                    den_cols = smallp.tile([P, it_n], F32, name="den_cols")
                    nc.sync.dma_start(
                        out=den_cols[:, :],
                        in_=den_dram.rearrange("o (t p) -> (o p) t", p=P),
                    )
                    recip = smallp.tile([P, it_n], F32, name="recip")
                    nc.vector.reciprocal(out=recip[:, :], in_=den_cols[:, :])

                    # ---- phase 2: out = P @ E_block, scaled by 1/den ----
                    for gb in range(gb_n):
                        ps_list = [
                            mmpsum.tile([P, 512], F32, name=f"ps{it}",
                                        tag="mm")
                            for it in range(it_n)
                        ]
                        for j in range(jt_n):
                            e_sb = epool.tile([P, 512], mm_dt, name="e_sb")
                            nc.sync.dma_start(
                                out=e_sb[:, :],
                                in_=e_d[j * P:(j + 1) * P,
                                        gb * 512:(gb + 1) * 512].bitcast(mm_dt),
                            )
                            for it in range(it_n):
                                nc.tensor.matmul(
                                    ps_list[it][:, :],
                                    pt[:, j, it * P:(it + 1) * P],
                                    e_sb[:, :],
                                    start=(j == 0), stop=(j == jt_n - 1),
                                )
                        for it in range(it_n):
                            o_sb = opool.tile([P, 512], F32, name="o_sb")
                            nc.vector.tensor_scalar_mul(
                                out=o_sb[:, :], in0=ps_list[it][:, :],
                                scalar1=recip[:, it:it + 1],
                            )
                            nc.sync.dma_start(
                                out=o_d[h * half + it * P:
                                        h * half + (it + 1) * P,
                                        gb * 512:(gb + 1) * 512],
                                in_=o_sb[:, :],
                            )

            if hw_loop:
                with tc.For_i(0, hw_loop, 1):
                    body()
            else:
                body()

    nc.compile()
    return nc


def make_in_maps(expression, encoding, quality, n_cores=N_CORES):
    b, n, d = encoding.shape
    g = expression.shape[2]
    rows = n // n_cores
    enc = np.ascontiguousarray(np.asarray(encoding, dtype=np.float32)[0])
    q = np.ascontiguousarray(np.asarray(quality, dtype=np.float32)[0, :, 0])
    expr = np.ascontiguousarray(np.asarray(expression, dtype=np.float32)[0])

    x2 = (enc.astype(np.float64) ** 2).sum(axis=1).astype(np.float32)
    k = d + 2
    u = np.empty((k, n), np.float32)
    u[:d] = enc.T
    u[d] = x2
    u[d + 1] = 1.0
    v_all = np.empty((k, n), np.float32)
    v_all[:d] = -2.0 * enc.T
    v_all[d] = 1.0
    v_all[d + 1] = x2

    # Per-core j-rotation: roll the j-indexed inputs by -rows*c so each
    # core's diagonal block sits at the same compile-time j-tiles on every
    # core (softmax's sum over j is permutation invariant, so the output is
    # unchanged). v is i-indexed and is not rolled.
    in_maps = []
    for c in range(n_cores):
        sh = -(c * rows)
        qc = np.roll(q, sh)
        in_maps.append({
            "u": np.ascontiguousarray(np.roll(u, sh, axis=1)),
            "v": np.ascontiguousarray(v_all[:, c * rows:(c + 1) * rows]),
            "qt": np.ascontiguousarray(qc.reshape(n // P, P).T),
            "expr": np.ascontiguousarray(np.roll(expr, sh, axis=0)),
        })
    return in_maps


_NC_CACHE = {}


def _get_nc(n, d, rows, g, use_f32r=True, repeat=1, hw_loop=0):
    key = (n, d, rows, g, use_f32r, repeat, hw_loop)
    if key not in _NC_CACHE:
        _NC_CACHE[key] = build_nc(n=n, d=d, rows=rows, g=g, use_f32r=use_f32r,
                                  repeat=repeat, hw_loop=hw_loop)
    return _NC_CACHE[key]


def kernel(expression, encoding, quality):
    from concourse.bass_utils import run_bass_kernel_spmd

    expression = np.asarray(expression)
    encoding = np.asarray(encoding)
    quality = np.asarray(quality)
    b, n, d = encoding.shape
    g = expression.shape[2]
    rows = n // N_CORES

    nc = _get_nc(n, d, rows, g)
    in_maps = make_in_maps(expression, encoding, quality)
    res = run_bass_kernel_spmd(nc, in_maps, core_ids=list(range(N_CORES)))
    out = np.concatenate([res.results[c]["out"] for c in range(N_CORES)], axis=0)
    return out[None].astype(np.float32)
